# revision 24
# baseline (speedup 1.0000x reference)
"""Trainium2 Bass kernel for nn_BlockLinear forward.

Computes y[b, o] = sum_k exp(log_weight[o, k]) * x[b, o*K + k]
for x [16384, 8192], log_weight [1024, 8] (fp32 interface).

Strategy: data-parallel over batch across 8 NeuronCores (2048 rows each),
fp16 on the wire (the 2e-2 rel-err gate leaves ~40x headroom over fp16's
~5e-4).  Host casts x to fp16 and sends w=exp(log_weight) as [1, 8192]
fp16; the device streams 16 tiles of [128, 8192] per core and runs ONE
custom DVE instruction per tile:

    y[p, g] = sum_k x[p, 8g+k] * w[p, 8g+k]

The custom op (SEGSUM8_PAIR2X_ANT) is a COUNT-driven uop FSM (K=8 is
static; repeat_count counts issue cycles) with both a 1x program and a
hand-packed 2x_1PORT program.  In 2x mode the DVE consumes two packed
fp16 elements per cycle (SRC/SRC_HI lanes): blocks 0/1 form the two
products, block 2 adds the pair, block 3 holds the running group sum in
its CURR_ALU_OUT flop (II=1).  Group sums are emitted in PAIRS: the even
group's sum is captured into block 3's delay-4 flop (which HOLDS while
later uops leave the lane disabled there) during the odd group's reset
uop, and the odd group's last pair writes {WR0_LO=even, WR0_HI=odd} -
one aligned 4-byte fp16x2 store per 8 cycles, satisfying the 2x dst
preconditions (dense, step 1, 2B dtype).  Both programs HW-verified
(synthetic group patterns, exact match).

Engine budget per tile: DVE 4.4-5.3us (2x; ~2 bubble cycles per 8-cycle
uop loop) vs DMA 6.5us (2 MiB x + 0.25 MiB y at ~358 GB/s HBM-per-NC) ->
memory-bound at the fp16 roofline.  Loads ride the Sync HWDGE queue,
stores the ScalarE HWDGE queue (FIFO per engine, so store sem-waits never
block load issue).  w [1, 8192] (16 KiB) loads first on the Sync FIFO and
is broadcast to all 128 partitions OFF the HBM stream by the idle PE
(ones[1,128] matmul into PSUM chunks) + ACT (PSUM->SBUF fp16 copies),
saving the 2 MiB a host-replicated w would cost; x tile 0 is HALF-split
(quarters=2) so its scans gate on per-half wb ranges - halves beat both
4-way splits (more issue slots/sem traffic at the head) and no split
(first scan gates on the whole wb broadcast): 109.0 vs 115.6 vs 125.2us
same-epoch minima; the last tile is quarter-split to shorten the drain.

Measured on the 8 axon trn2 cores: HW exec 109.0-128.5us across runs
(fresh-process first runs land ~109us, 3/3 observed; later back-to-back
runs ~127us - the spread is HBM rate, 330-380 GB/s, not pipeline gaps:
the DMA-busy timeline is continuous from ~8us to end-of-stream), vs
212.5us for the fp32 baseline at the same rel err gate.  Per-core
stream: 36 MiB + ~7.3us framework preamble + ~4us epilogue.  rpt=2
double tiles (4 MiB loads, 32 KiB/partition bursts) measured
consistently ~7us WORSE - keep rpt=1.  Scale-relative error 5.2e-4
(fp16 in/out, fp32 accumulate), gate 2e-2.
"""

import numpy as np

B = 16384
IN_F = 8192
OUT_F = 1024
K = 8
N_CORES = 8
P = 128

_CACHE = {}

_OP_NAME = "SEGSUM8_PAIR2X_ANT"


def _build_pair_uops():
    """Build (uops_1x, uops_2x) for the grouped (K=8) multiply-reduce.

    Both programs are COUNT-driven (no SUB_DIM triggers): the group
    structure is static, so the FSM loops on element counts.  next_uop
    index 0 means IDLE, so the steady-state loop lives at indices >= 1
    and the entry uop at index 0 is a one-shot copy of the loop head.
    """
    from concourse.dve_uop import (
        ENABLE,
        AluInp,
        AluOp,
        DelayInp,
        InpSel,
        OutPath,
        OutSel,
        Trigger,
        UopConfig,
        UopDpConfig,
    )

    def chain(u, rep, nxt):
        u.repeat_count = rep
        u.trigger = (Trigger.SRC_TENSOR_DONE, Trigger.COUNT, Trigger.NONE)
        u.next_uop = (0, nxt, 0)
        u.require_inp0 = ENABLE
        u.require_inp1 = ENABLE
        return u

    # ---- 1x program: 1 elem/cycle, acc in block1, write once per group ----
    def u1x(kind, write):
        u = UopConfig()
        u.enable_input(InpSel.SRC_0, 0)  # x   -> block0 ALU A
        u.enable_input(InpSel.SRC_1, 1)  # w   -> block0 PREV_DELAY_0
        u.datapath_config[0] = UopDpConfig().enable_alu(
            AluOp.MULTIPLY, AluInp.PREV_ALU_OUT, AluInp.PREV_DELAY_0
        )
        op = AluOp.BYPASS if kind == "reset" else AluOp.ADD
        u.datapath_config[1] = UopDpConfig().enable_alu(
            op, AluInp.PREV_ALU_OUT, AluInp.CURR_ALU_OUT
        )
        for b in range(2, 8):
            u.datapath_config[b] = UopDpConfig().pass_through_alu()
        if write:
            u.enable_output(OutSel.ALU_OUT, OutPath.WR0_LO)
        return u

    # reset(1) -> acc(6) -> emit(1,write) -> reset(1) -> acc(6) -> emit -> ...
    ops_1x = [
        chain(u1x("reset", False), 1, 1),  # 0: entry reset
        chain(u1x("acc", False), 6, 2),    # 1
        chain(u1x("acc", True), 1, 3),     # 2: emit (8th element completes sum)
        chain(u1x("reset", False), 1, 4),  # 3: loop reset
        chain(u1x("acc", False), 6, 5),    # 4
        chain(u1x("acc", True), 1, 3),     # 5: emit -> loop reset
    ]

    # ---- 2x_1PORT program: 2 packed fp16/cycle, acc in block3 --------------
    def u2x(kind):
        u = UopConfig()
        u.enable_input(InpSel.SRC_0, 0)     # x_lo -> block0 ALU A
        u.enable_input(InpSel.SRC_1, 1)     # w_lo -> PREV_DELAY_0
        u.enable_input(InpSel.SRC_0_HI, 2)  # x_hi -> PREV_DELAY_1
        u.enable_input(InpSel.SRC_1_HI, 3)  # w_hi -> PREV_DELAY_2
        u.datapath_config[0] = (
            UopDpConfig()
            .enable_alu(AluOp.MULTIPLY, AluInp.PREV_ALU_OUT, AluInp.PREV_DELAY_0)
            .pass_through_delay(1, 2)
        )
        u.datapath_config[1] = (
            UopDpConfig()
            .enable_alu(AluOp.MULTIPLY, AluInp.PREV_DELAY_1, AluInp.PREV_DELAY_2)
            .enable_delay_from_src(DelayInp.PREV_ALU_OUT, 0)  # m_lo
        )
        u.datapath_config[2] = UopDpConfig().enable_alu(
            AluOp.ADD, AluInp.PREV_ALU_OUT, AluInp.PREV_DELAY_0  # t = m_hi+m_lo
        )
        b3 = UopDpConfig()
        if kind in ("reset", "reset_hold"):
            # drop the CURR feedback: acc <- t (new group's first pair)
            b3.enable_alu(AluOp.BYPASS, AluInp.PREV_ALU_OUT, AluInp.CURR_ALU_OUT)
            if kind == "reset_hold":
                # capture the completed even-group sum (block3's CURR flop)
                # into delay lane 4; later uops leave d4 disabled at block3,
                # so the flop HOLDS it until the emit element reads it.
                b3.enable_delay_from_src(DelayInp.CURR_ALU_OUT, 4)
        else:
            b3.enable_alu(AluOp.ADD, AluInp.PREV_ALU_OUT, AluInp.CURR_ALU_OUT)
        u.datapath_config[3] = b3
        for b in range(4, 8):
            cfg = UopDpConfig().pass_through_alu()
            if kind in ("emit", "acc", "reset_hold"):
                # keep the held even sum flowing toward the output flops
                cfg.pass_through_delay(4)
            u.datapath_config[b] = cfg
        if kind == "emit":
            # one packed 4B write per pair of groups: {lo=even sum, hi=odd sum}
            u.enable_output(OutSel.DELAY_4, OutPath.WR0_LO)
            u.enable_output(OutSel.ALU_OUT, OutPath.WR0_HI)
        return u

    # repeat_count counts ISSUE CYCLES (2 elements each in 2x mode):
    # resetE(1) -> accA(3) -> resetO(1, hold even sum) -> accB(2)
    #   -> emit(1, write pair) -> resetE(1) -> accA ...
    ops_2x = [
        chain(u2x("reset"), 1, 1),       # 0: entry (even group pair 0)
        chain(u2x("acc"), 3, 2),         # 1: even pairs 1-3
        chain(u2x("reset_hold"), 1, 3),  # 2: odd pair 0, park even sum in d4
        chain(u2x("acc"), 2, 4),         # 3: odd pairs 1-2
        chain(u2x("emit"), 1, 5),        # 4: odd pair 3, write {even,odd}
        chain(u2x("reset"), 1, 1),       # 5: loop reset (next even group)
    ]
    return ops_1x, ops_2x


def _register_pair_op():
    """Register SEGSUM8_PAIR2X_ANT (1x + 2x_1PORT programs, perf_max=1)."""
    import dataclasses

    from concourse import dve_ops
    from concourse.dve_spec import AluOp, Spec, Src0, Src1, scan
    from concourse.dve_uop import DveOpSpec

    for op in dve_ops.OPS:
        if op.name == _OP_NAME:
            return op

    def _ref(in0, in1, s0, s1, imm2):
        p = np.asarray(in0, np.float32) * np.asarray(in1, np.float32).reshape(
            np.asarray(in0).shape
        )
        return p.reshape(p.shape[0], -1, K).sum(axis=-1, dtype=np.float32)

    # body is structural only (Src1 presence -> rd1_en); semantics live in
    # the hand-built uop programs + `_ref` (used by the interpreter).
    spec = Spec(body=scan(AluOp.ADD, Src0 * Src1), reference=_ref)

    @dataclasses.dataclass(frozen=True)
    class _PairDveOp(dve_ops.DveOp):
        def compile(self, ver):
            key = (self.name, ver)
            cached = dve_ops._COMPILE_CACHE.get(key)
            if cached is not None:
                return cached
            u1x, u2x = _build_pair_uops()
            result = DveOpSpec(
                name=self.name,
                opcode=dve_ops.get_dve_sub_opcode(self.name),
                uops=u1x,
                uops_2x=u2x,
                perf_max=1,
                rd1_en=True,
            )
            result.validate(ver)
            dve_ops._COMPILE_CACHE[key] = result
            return result

    row = dve_ops._CUSTOM_DVE_ROW_BASE + len(dve_ops.OPS)
    op = _PairDveOp(_OP_NAME, spec, subdim=False, uops_sha={})
    dve_ops.OPS.append(op)
    dve_ops.CUSTOM_DVE_SPECS[_OP_NAME] = spec
    dve_ops._SUB_OPCODE_FOR_NAME[_OP_NAME] = row
    return op


def _build(b_shard, in_f, out_f, n_cores, x_bufs=4, quarters=2, tail_quarters=2,
           rpt=1):
    """Build + compile the per-core Bass module (SPMD across n_cores).

    rpt = x rows per partition in the steady-stream tiles.  rpt=2 makes
    each load a [128, 2, 8192] AP (4 MiB, 32 KiB contiguous per
    partition) - longer HBM bursts that hold rate better under
    paired-NC contention; w is then replicated rpt times along the free
    dim of wb so one scan instruction covers the whole tile stream.
    """
    from concourse import bacc, mybir, tile

    op = _register_pair_op()

    k = K
    n_blocks = b_shard // P
    qw = in_f // quarters  # quarter width (multiple of 16)
    f16 = mybir.dt.float16
    assert (n_blocks - 2) % rpt == 0, (n_blocks, rpt)

    nc = bacc.Bacc(
        "TRN2",
        target_bir_lowering=False,
        debug=False,
        enable_asserts=True,
        num_devices=n_cores,
    )
    x_d = nc.dram_tensor("x", [b_shard, in_f], f16, kind="ExternalInput")
    w_d = nc.dram_tensor("w", [1, in_f], f16, kind="ExternalInput")
    y_d = nc.dram_tensor("y", [b_shard, out_f], f16, kind="ExternalOutput")

    with tile.TileContext(nc) as tc:
        with (
            tc.tile_pool(name="consts", bufs=1) as cpool,
            tc.tile_pool(name="first", bufs=1) as fpool,
            tc.tile_pool(name="work", bufs=x_bufs) as pool,
            tc.tile_pool(name="outs", bufs=3) as ypool,
            tc.tile_pool(name="tailq", bufs=4) as qpool,
            tc.psum_pool(name="wpsum", bufs=2) as ppool,
        ):
            wb = cpool.tile([P, rpt * in_f], f16, tag="w")
            w_sb = cpool.tile([1, in_f], f16, tag="w_row")
            ones = cpool.tile([1, P], f16, tag="ones")

            def chunk(xap, w0, cw, y_ap, y_rearrange=None):
                """Grouped multiply-reduce of xap against wb[:, w0:w0+cw]."""
                cg = cw // k
                yt = ypool.tile([P, cg], f16, tag="s")
                ins = nc.vector._custom_dve(
                    op,
                    out=yt[:],
                    in0=xap,
                    in1=wb[:, w0 : w0 + cw],
                )
                ins.ins.perf_max = 1  # byte-36[7:6]: allow the 2x_1PORT slot
                src = yt[:]
                if y_rearrange is not None:
                    src = src.rearrange(y_rearrange, g=out_f)
                # y stores ride the ScalarE HWDGE queue so their semaphore
                # waits never block the x-load issue stream.
                nc.scalar.dma_start(out=y_ap, in_=src)

            # w broadcast [1, in_f] -> [128, rpt*in_f] OFF the HBM stream:
            # a 16 KiB w load rides first on the Sync FIFO, then the (idle)
            # PE replicates it into PSUM chunks via a ones[1,128] matmul
            # (K=1 contraction) and the (idle) ACT engine copies each PSUM
            # chunk to wb as fp16 (w repeated rpt times along the free
            # dim).  Saves the 2 MiB+ a host-replicated w load would cost.
            nc.sync.dma_start(out=w_sb[:], in_=w_d[:])
            nc.vector.memset(ones[:], 1.0)
            mm = 512  # PE moving-free-dim max; [P, 512] fp32 = 1 PSUM bank
            for c in range(rpt * in_f // mm):
                s0 = (c * mm) % in_f
                pt = ppool.tile([P, mm], mybir.dt.float32, tag="wp")
                nc.tensor.matmul(
                    pt[:], ones[0:1, :], w_sb[0:1, s0 : s0 + mm],
                    start=True, stop=True,
                )
                nc.scalar.copy(out=wb[:, c * mm : (c + 1) * mm], in_=pt[:])

            # block 0 in quarters: quarter-scan q gates only on its own wb
            # range + x0 quarter, so scans start early.
            xt0 = fpool.tile([P, in_f], f16, tag="x0")
            for q in range(quarters):
                cs = slice(q * qw, (q + 1) * qw)
                nc.sync.dma_start(out=xt0[:, cs], in_=x_d[0:P, cs])
                chunk(xt0[:, cs], q * qw, qw, y_d[0:P, q * qw // k : (q + 1) * qw // k])

            # steady stream: blocks 1 .. n_blocks-2, rpt row-blocks per tile
            for r in range(P, (n_blocks - 1) * P, rpt * P):
                xt = pool.tile([P, rpt * in_f], f16, tag="x")
                nc.sync.dma_start(
                    out=xt[:].rearrange("p (t f) -> p t f", f=in_f),
                    in_=x_d[r : r + rpt * P, :].rearrange(
                        "(p t) f -> p t f", t=rpt
                    ),
                )
                chunk(
                    xt[:],
                    0,
                    rpt * in_f,
                    y_d[r : r + rpt * P, :].rearrange("(p t) g -> p t g", t=rpt),
                    y_rearrange="p (t g) -> p t g",
                )

            # split the final block so the post-stream tail is short
            rows = slice((n_blocks - 1) * P, n_blocks * P)
            tqw = in_f // tail_quarters
            for q in range(tail_quarters):
                xq = qpool.tile([P, tqw], f16, tag="xq")
                nc.sync.dma_start(
                    out=xq[:], in_=x_d[rows, q * tqw : (q + 1) * tqw]
                )
                chunk(xq[:], q * tqw, tqw,
                      y_d[rows, q * tqw // k : (q + 1) * tqw // k])
    nc.compile()
    return nc


def _prep_weights(log_weight):
    w = np.exp(np.asarray(log_weight, np.float64)).reshape(1, -1)  # [1, out_f*k]
    return np.ascontiguousarray(w.astype(np.float16))


def kernel(x, log_weight):
    from concourse import bass_utils

    x = np.ascontiguousarray(np.asarray(x).astype(np.float16))
    assert x.shape == (B, IN_F), x.shape
    b_shard = B // N_CORES

    if "nc" not in _CACHE:
        _CACHE["nc"] = _build(b_shard, IN_F, OUT_F, N_CORES)
    nc = _CACHE["nc"]

    wb = _prep_weights(log_weight)
    in_maps = [
        {"x": x[i * b_shard : (i + 1) * b_shard], "w": wb}
        for i in range(N_CORES)
    ]
    res = bass_utils.run_bass_kernel_spmd(nc, in_maps, core_ids=list(range(N_CORES)))
    y = np.concatenate([res.results[i]["y"] for i in range(N_CORES)], axis=0)
    return y.astype(np.float32)


# revision 26
# speedup vs baseline: 1.0381x; 1.0381x over previous
"""Trainium2 Bass kernel for nn_BlockLinear forward.

Computes y[b, o] = sum_k exp(log_weight[o, k]) * x[b, o*K + k]
for x [16384, 8192], log_weight [1024, 8] (fp32 interface).

Strategy: data-parallel over batch across 8 NeuronCores (2048 rows each),
fp16 on the wire (the 2e-2 rel-err gate leaves ~40x headroom over fp16's
~5e-4).  Host casts x to fp16 and sends w=exp(log_weight) as [1, 8192]
fp16; the device streams 16 tiles of [128, 8192] per core and runs ONE
custom DVE instruction per tile:

    y[p, g] = sum_k x[p, 8g+k] * w[p, 8g+k]

The custom op (SEGSUM8_PAIR2X_ANT) is a COUNT-driven uop FSM (K=8 is
static; repeat_count counts issue cycles) with both a 1x program and a
hand-packed 2x_1PORT program.  In 2x mode the DVE consumes two packed
fp16 elements per cycle (SRC/SRC_HI lanes): blocks 0/1 form the two
products, block 2 adds the pair, block 3 holds the running group sum in
its CURR_ALU_OUT flop (II=1).  Group sums are emitted in PAIRS: the even
group's sum is captured into block 3's delay-4 flop (which HOLDS while
later uops leave the lane disabled there) during the odd group's reset
uop, and the odd group's last pair writes {WR0_LO=even, WR0_HI=odd} -
one aligned 4-byte fp16x2 store per 8 cycles, satisfying the 2x dst
preconditions (dense, step 1, 2B dtype).  Both programs HW-verified
(synthetic group patterns, exact match).

Engine budget per tile: DVE 4.4-5.3us (2x; ~2 bubble cycles per 8-cycle
uop loop) vs DMA 6.5us (2 MiB x + 0.25 MiB y at ~358 GB/s HBM-per-NC) ->
memory-bound at the fp16 roofline.  Loads ride the Sync HWDGE queue,
stores the ScalarE HWDGE queue (FIFO per engine, so store sem-waits never
block load issue).  w [1, 8192] (16 KiB) loads first on the Sync FIFO and
is broadcast to all 128 partitions OFF the HBM stream by the idle PE
(ones[1,128] matmul into PSUM chunks) + ACT (PSUM->SBUF fp16 copies),
saving the 2 MiB a host-replicated w would cost; x tile 0 is HALF-split
(quarters=2) so its scans gate on per-half wb ranges - halves beat both
4-way splits (more issue slots/sem traffic at the head) and no split
(first scan gates on the whole wb broadcast): 109.0 vs 115.6 vs 125.2us
same-epoch minima.  The last tile is HALF-split too (tail_quarters=2
beat 4 and 1: 108.3 vs 109.0 vs 110.6us minima, and 3/5 vs 1/5 vs 2/5
runs landing in the fast regime) - split granularity 2 is the sweet
spot at both ends of the stream.

Measured on the 8 axon trn2 cores: HW exec 108.3-128.5us across runs
(fresh-process early runs land ~108-110us; later back-to-back runs
~127us - the spread is HBM rate, 330-390 GB/s, not pipeline gaps: the
DMA-busy timeline is continuous from ~8us to end-of-stream), vs 212.5us
for the fp32 baseline at the same rel err gate.  Per-core stream:
36 MiB + ~7.3us framework preamble + ~4us epilogue.  rpt=2 double tiles
(4 MiB loads, 32 KiB/partition bursts) measured consistently ~7us
WORSE - keep rpt=1.  Scale-relative error 5.2e-4 (fp16 in/out, fp32
accumulate), gate 2e-2.
"""

import numpy as np

B = 16384
IN_F = 8192
OUT_F = 1024
K = 8
N_CORES = 8
P = 128

_CACHE = {}

_OP_NAME = "SEGSUM8_PAIR2X_ANT"


def _build_pair_uops():
    """Build (uops_1x, uops_2x) for the grouped (K=8) multiply-reduce.

    Both programs are COUNT-driven (no SUB_DIM triggers): the group
    structure is static, so the FSM loops on element counts.  next_uop
    index 0 means IDLE, so the steady-state loop lives at indices >= 1
    and the entry uop at index 0 is a one-shot copy of the loop head.
    """
    from concourse.dve_uop import (
        ENABLE,
        AluInp,
        AluOp,
        DelayInp,
        InpSel,
        OutPath,
        OutSel,
        Trigger,
        UopConfig,
        UopDpConfig,
    )

    def chain(u, rep, nxt):
        u.repeat_count = rep
        u.trigger = (Trigger.SRC_TENSOR_DONE, Trigger.COUNT, Trigger.NONE)
        u.next_uop = (0, nxt, 0)
        u.require_inp0 = ENABLE
        u.require_inp1 = ENABLE
        return u

    # ---- 1x program: 1 elem/cycle, acc in block1, write once per group ----
    def u1x(kind, write):
        u = UopConfig()
        u.enable_input(InpSel.SRC_0, 0)  # x   -> block0 ALU A
        u.enable_input(InpSel.SRC_1, 1)  # w   -> block0 PREV_DELAY_0
        u.datapath_config[0] = UopDpConfig().enable_alu(
            AluOp.MULTIPLY, AluInp.PREV_ALU_OUT, AluInp.PREV_DELAY_0
        )
        op = AluOp.BYPASS if kind == "reset" else AluOp.ADD
        u.datapath_config[1] = UopDpConfig().enable_alu(
            op, AluInp.PREV_ALU_OUT, AluInp.CURR_ALU_OUT
        )
        for b in range(2, 8):
            u.datapath_config[b] = UopDpConfig().pass_through_alu()
        if write:
            u.enable_output(OutSel.ALU_OUT, OutPath.WR0_LO)
        return u

    # reset(1) -> acc(6) -> emit(1,write) -> reset(1) -> acc(6) -> emit -> ...
    ops_1x = [
        chain(u1x("reset", False), 1, 1),  # 0: entry reset
        chain(u1x("acc", False), 6, 2),    # 1
        chain(u1x("acc", True), 1, 3),     # 2: emit (8th element completes sum)
        chain(u1x("reset", False), 1, 4),  # 3: loop reset
        chain(u1x("acc", False), 6, 5),    # 4
        chain(u1x("acc", True), 1, 3),     # 5: emit -> loop reset
    ]

    # ---- 2x_1PORT program: 2 packed fp16/cycle, acc in block3 --------------
    def u2x(kind):
        u = UopConfig()
        u.enable_input(InpSel.SRC_0, 0)     # x_lo -> block0 ALU A
        u.enable_input(InpSel.SRC_1, 1)     # w_lo -> PREV_DELAY_0
        u.enable_input(InpSel.SRC_0_HI, 2)  # x_hi -> PREV_DELAY_1
        u.enable_input(InpSel.SRC_1_HI, 3)  # w_hi -> PREV_DELAY_2
        u.datapath_config[0] = (
            UopDpConfig()
            .enable_alu(AluOp.MULTIPLY, AluInp.PREV_ALU_OUT, AluInp.PREV_DELAY_0)
            .pass_through_delay(1, 2)
        )
        u.datapath_config[1] = (
            UopDpConfig()
            .enable_alu(AluOp.MULTIPLY, AluInp.PREV_DELAY_1, AluInp.PREV_DELAY_2)
            .enable_delay_from_src(DelayInp.PREV_ALU_OUT, 0)  # m_lo
        )
        u.datapath_config[2] = UopDpConfig().enable_alu(
            AluOp.ADD, AluInp.PREV_ALU_OUT, AluInp.PREV_DELAY_0  # t = m_hi+m_lo
        )
        b3 = UopDpConfig()
        if kind in ("reset", "reset_hold"):
            # drop the CURR feedback: acc <- t (new group's first pair)
            b3.enable_alu(AluOp.BYPASS, AluInp.PREV_ALU_OUT, AluInp.CURR_ALU_OUT)
            if kind == "reset_hold":
                # capture the completed even-group sum (block3's CURR flop)
                # into delay lane 4; later uops leave d4 disabled at block3,
                # so the flop HOLDS it until the emit element reads it.
                b3.enable_delay_from_src(DelayInp.CURR_ALU_OUT, 4)
        else:
            b3.enable_alu(AluOp.ADD, AluInp.PREV_ALU_OUT, AluInp.CURR_ALU_OUT)
        u.datapath_config[3] = b3
        for b in range(4, 8):
            cfg = UopDpConfig().pass_through_alu()
            if kind in ("emit", "acc", "reset_hold"):
                # keep the held even sum flowing toward the output flops
                cfg.pass_through_delay(4)
            u.datapath_config[b] = cfg
        if kind == "emit":
            # one packed 4B write per pair of groups: {lo=even sum, hi=odd sum}
            u.enable_output(OutSel.DELAY_4, OutPath.WR0_LO)
            u.enable_output(OutSel.ALU_OUT, OutPath.WR0_HI)
        return u

    # repeat_count counts ISSUE CYCLES (2 elements each in 2x mode):
    # resetE(1) -> accA(3) -> resetO(1, hold even sum) -> accB(2)
    #   -> emit(1, write pair) -> resetE(1) -> accA ...
    ops_2x = [
        chain(u2x("reset"), 1, 1),       # 0: entry (even group pair 0)
        chain(u2x("acc"), 3, 2),         # 1: even pairs 1-3
        chain(u2x("reset_hold"), 1, 3),  # 2: odd pair 0, park even sum in d4
        chain(u2x("acc"), 2, 4),         # 3: odd pairs 1-2
        chain(u2x("emit"), 1, 5),        # 4: odd pair 3, write {even,odd}
        chain(u2x("reset"), 1, 1),       # 5: loop reset (next even group)
    ]
    return ops_1x, ops_2x


def _register_pair_op():
    """Register SEGSUM8_PAIR2X_ANT (1x + 2x_1PORT programs, perf_max=1)."""
    import dataclasses

    from concourse import dve_ops
    from concourse.dve_spec import AluOp, Spec, Src0, Src1, scan
    from concourse.dve_uop import DveOpSpec

    for op in dve_ops.OPS:
        if op.name == _OP_NAME:
            return op

    def _ref(in0, in1, s0, s1, imm2):
        p = np.asarray(in0, np.float32) * np.asarray(in1, np.float32).reshape(
            np.asarray(in0).shape
        )
        return p.reshape(p.shape[0], -1, K).sum(axis=-1, dtype=np.float32)

    # body is structural only (Src1 presence -> rd1_en); semantics live in
    # the hand-built uop programs + `_ref` (used by the interpreter).
    spec = Spec(body=scan(AluOp.ADD, Src0 * Src1), reference=_ref)

    @dataclasses.dataclass(frozen=True)
    class _PairDveOp(dve_ops.DveOp):
        def compile(self, ver):
            key = (self.name, ver)
            cached = dve_ops._COMPILE_CACHE.get(key)
            if cached is not None:
                return cached
            u1x, u2x = _build_pair_uops()
            result = DveOpSpec(
                name=self.name,
                opcode=dve_ops.get_dve_sub_opcode(self.name),
                uops=u1x,
                uops_2x=u2x,
                perf_max=1,
                rd1_en=True,
            )
            result.validate(ver)
            dve_ops._COMPILE_CACHE[key] = result
            return result

    row = dve_ops._CUSTOM_DVE_ROW_BASE + len(dve_ops.OPS)
    op = _PairDveOp(_OP_NAME, spec, subdim=False, uops_sha={})
    dve_ops.OPS.append(op)
    dve_ops.CUSTOM_DVE_SPECS[_OP_NAME] = spec
    dve_ops._SUB_OPCODE_FOR_NAME[_OP_NAME] = row
    return op


def _build(b_shard, in_f, out_f, n_cores, x_bufs=4, quarters=2, tail_quarters=2,
           rpt=1):
    """Build + compile the per-core Bass module (SPMD across n_cores).

    rpt = x rows per partition in the steady-stream tiles.  rpt=2 makes
    each load a [128, 2, 8192] AP (4 MiB, 32 KiB contiguous per
    partition) - longer HBM bursts that hold rate better under
    paired-NC contention; w is then replicated rpt times along the free
    dim of wb so one scan instruction covers the whole tile stream.
    """
    from concourse import bacc, mybir, tile

    op = _register_pair_op()

    k = K
    n_blocks = b_shard // P
    qw = in_f // quarters  # quarter width (multiple of 16)
    f16 = mybir.dt.float16
    assert (n_blocks - 2) % rpt == 0, (n_blocks, rpt)

    nc = bacc.Bacc(
        "TRN2",
        target_bir_lowering=False,
        debug=False,
        enable_asserts=True,
        num_devices=n_cores,
    )
    x_d = nc.dram_tensor("x", [b_shard, in_f], f16, kind="ExternalInput")
    w_d = nc.dram_tensor("w", [1, in_f], f16, kind="ExternalInput")
    y_d = nc.dram_tensor("y", [b_shard, out_f], f16, kind="ExternalOutput")

    with tile.TileContext(nc) as tc:
        with (
            tc.tile_pool(name="consts", bufs=1) as cpool,
            tc.tile_pool(name="first", bufs=1) as fpool,
            tc.tile_pool(name="work", bufs=x_bufs) as pool,
            tc.tile_pool(name="outs", bufs=3) as ypool,
            tc.tile_pool(name="tailq", bufs=4) as qpool,
            tc.psum_pool(name="wpsum", bufs=2) as ppool,
        ):
            wb = cpool.tile([P, rpt * in_f], f16, tag="w")
            w_sb = cpool.tile([1, in_f], f16, tag="w_row")
            ones = cpool.tile([1, P], f16, tag="ones")

            def chunk(xap, w0, cw, y_ap, y_rearrange=None):
                """Grouped multiply-reduce of xap against wb[:, w0:w0+cw]."""
                cg = cw // k
                yt = ypool.tile([P, cg], f16, tag="s")
                ins = nc.vector._custom_dve(
                    op,
                    out=yt[:],
                    in0=xap,
                    in1=wb[:, w0 : w0 + cw],
                )
                ins.ins.perf_max = 1  # byte-36[7:6]: allow the 2x_1PORT slot
                src = yt[:]
                if y_rearrange is not None:
                    src = src.rearrange(y_rearrange, g=out_f)
                # y stores ride the ScalarE HWDGE queue so their semaphore
                # waits never block the x-load issue stream.
                nc.scalar.dma_start(out=y_ap, in_=src)

            # w broadcast [1, in_f] -> [128, rpt*in_f] OFF the HBM stream:
            # a 16 KiB w load rides first on the Sync FIFO, then the (idle)
            # PE replicates it into PSUM chunks via a ones[1,128] matmul
            # (K=1 contraction) and the (idle) ACT engine copies each PSUM
            # chunk to wb as fp16 (w repeated rpt times along the free
            # dim).  Saves the 2 MiB+ a host-replicated w load would cost.
            nc.sync.dma_start(out=w_sb[:], in_=w_d[:])
            nc.vector.memset(ones[:], 1.0)
            mm = 512   # PE moving-free-dim max; [P, 512] fp32 = 1 PSUM bank
            cw2 = 1024  # 2 banks per ACT copy: halves the op/sem traffic
            for c in range(rpt * in_f // cw2):
                pt = ppool.tile([P, cw2], mybir.dt.float32, tag="wp")
                for h in range(cw2 // mm):
                    s0 = (c * cw2 + h * mm) % in_f
                    nc.tensor.matmul(
                        pt[:, h * mm : (h + 1) * mm], ones[0:1, :],
                        w_sb[0:1, s0 : s0 + mm], start=True, stop=True,
                    )
                nc.scalar.copy(out=wb[:, c * cw2 : (c + 1) * cw2], in_=pt[:])

            # block 0 in quarters: quarter-scan q gates only on its own wb
            # range + x0 quarter, so scans start early.
            xt0 = fpool.tile([P, in_f], f16, tag="x0")
            for q in range(quarters):
                cs = slice(q * qw, (q + 1) * qw)
                nc.sync.dma_start(out=xt0[:, cs], in_=x_d[0:P, cs])
                chunk(xt0[:, cs], q * qw, qw, y_d[0:P, q * qw // k : (q + 1) * qw // k])

            # steady stream: blocks 1 .. n_blocks-2, rpt row-blocks per tile
            for r in range(P, (n_blocks - 1) * P, rpt * P):
                xt = pool.tile([P, rpt * in_f], f16, tag="x")
                nc.sync.dma_start(
                    out=xt[:].rearrange("p (t f) -> p t f", f=in_f),
                    in_=x_d[r : r + rpt * P, :].rearrange(
                        "(p t) f -> p t f", t=rpt
                    ),
                )
                chunk(
                    xt[:],
                    0,
                    rpt * in_f,
                    y_d[r : r + rpt * P, :].rearrange("(p t) g -> p t g", t=rpt),
                    y_rearrange="p (t g) -> p t g",
                )

            # split the final block so the post-stream tail is short
            rows = slice((n_blocks - 1) * P, n_blocks * P)
            tqw = in_f // tail_quarters
            for q in range(tail_quarters):
                xq = qpool.tile([P, tqw], f16, tag="xq")
                nc.sync.dma_start(
                    out=xq[:], in_=x_d[rows, q * tqw : (q + 1) * tqw]
                )
                chunk(xq[:], q * tqw, tqw,
                      y_d[rows, q * tqw // k : (q + 1) * tqw // k])
    nc.compile()
    return nc


def _prep_weights(log_weight):
    w = np.exp(np.asarray(log_weight, np.float64)).reshape(1, -1)  # [1, out_f*k]
    return np.ascontiguousarray(w.astype(np.float16))


def kernel(x, log_weight):
    from concourse import bass_utils

    x = np.ascontiguousarray(np.asarray(x).astype(np.float16))
    assert x.shape == (B, IN_F), x.shape
    b_shard = B // N_CORES

    if "nc" not in _CACHE:
        _CACHE["nc"] = _build(b_shard, IN_F, OUT_F, N_CORES)
    nc = _CACHE["nc"]

    wb = _prep_weights(log_weight)
    in_maps = [
        {"x": x[i * b_shard : (i + 1) * b_shard], "w": wb}
        for i in range(N_CORES)
    ]
    res = bass_utils.run_bass_kernel_spmd(nc, in_maps, core_ids=list(range(N_CORES)))
    y = np.concatenate([res.results[i]["y"] for i in range(N_CORES)], axis=0)
    return y.astype(np.float32)


# revision 28
# speedup vs baseline: 1.0485x; 1.0100x over previous
"""Trainium2 Bass kernel for nn_BlockLinear forward.

Computes y[b, o] = sum_k exp(log_weight[o, k]) * x[b, o*K + k]
for x [16384, 8192], log_weight [1024, 8] (fp32 interface).

Strategy: data-parallel over batch across 8 NeuronCores (2048 rows each),
fp16 on the wire (the 2e-2 rel-err gate leaves ~40x headroom over fp16's
~5e-4).  Host casts x to fp16 and sends w=exp(log_weight) as [1, 8192]
fp16; the device streams 16 tiles of [128, 8192] per core and runs ONE
custom DVE instruction per tile:

    y[p, g] = sum_k x[p, 8g+k] * w[p, 8g+k]

The custom op (SEGSUM8_PAIR2X_ANT) is a COUNT-driven uop FSM (K=8 is
static; repeat_count counts issue cycles) with both a 1x program and a
hand-packed 2x_1PORT program.  In 2x mode the DVE consumes two packed
fp16 elements per cycle (SRC/SRC_HI lanes): blocks 0/1 form the two
products, block 2 adds the pair, block 3 holds the running group sum in
its CURR_ALU_OUT flop (II=1).  Group sums are emitted in PAIRS: the even
group's sum is captured into block 3's delay-4 flop (which HOLDS while
later uops leave the lane disabled there) during the odd group's reset
uop, and the odd group's last pair writes {WR0_LO=even, WR0_HI=odd} -
one aligned 4-byte fp16x2 store per 8 cycles, satisfying the 2x dst
preconditions (dense, step 1, 2B dtype).  Both programs HW-verified
(synthetic group patterns, exact match).

Engine budget per tile: DVE 4.4-5.3us (2x; ~2 bubble cycles per 8-cycle
uop loop) vs DMA 6.5us (2 MiB x + 0.25 MiB y at ~358 GB/s HBM-per-NC) ->
memory-bound at the fp16 roofline.  Loads ride the Sync HWDGE queue,
stores the ScalarE HWDGE queue (FIFO per engine, so store sem-waits never
block load issue).  w [1, 8192] (16 KiB) loads first on the Sync FIFO and
is broadcast to all 128 partitions OFF the HBM stream by the idle PE
(ones[1,128] matmul into PSUM chunks) + ACT (PSUM->SBUF fp16 copies),
saving the 2 MiB a host-replicated w would cost; x tile 0 is HALF-split
(quarters=2) so its scans gate on per-half wb ranges - halves beat both
4-way splits (more issue slots/sem traffic at the head) and no split
(first scan gates on the whole wb broadcast): 109.0 vs 115.6 vs 125.2us
same-epoch minima.  The last tile is HALF-split too (tail_quarters=2
beat 4 and 1: 108.3 vs 109.0 vs 110.6us minima, and 3/5 vs 1/5 vs 2/5
runs landing in the fast regime) - split granularity 2 is the sweet
spot at both ends of the stream.

Measured on the 8 axon trn2 cores: HW exec 106.5-128.5us across runs
(w-broadcast consolidated to 8 double-bank ACT copies: 106.5us best)
(fresh-process early runs land ~108-110us; later back-to-back runs
~127us - the spread is HBM rate, 330-390 GB/s, not pipeline gaps: the
DMA-busy timeline is continuous from ~8us to end-of-stream), vs 212.5us
for the fp32 baseline at the same rel err gate.  Per-core stream:
36 MiB + ~7.3us framework preamble + ~4us epilogue.  rpt=2 double tiles
(4 MiB loads, 32 KiB/partition bursts) measured consistently ~7us
WORSE - keep rpt=1.  Scale-relative error 5.2e-4 (fp16 in/out, fp32
accumulate), gate 2e-2.
"""

import numpy as np

B = 16384
IN_F = 8192
OUT_F = 1024
K = 8
N_CORES = 8
P = 128

_CACHE = {}

_OP_NAME = "SEGSUM8_PAIR2X_ANT"


def _build_pair_uops():
    """Build (uops_1x, uops_2x) for the grouped (K=8) multiply-reduce.

    Both programs are COUNT-driven (no SUB_DIM triggers): the group
    structure is static, so the FSM loops on element counts.  next_uop
    index 0 means IDLE, so the steady-state loop lives at indices >= 1
    and the entry uop at index 0 is a one-shot copy of the loop head.
    """
    from concourse.dve_uop import (
        ENABLE,
        AluInp,
        AluOp,
        DelayInp,
        InpSel,
        OutPath,
        OutSel,
        Trigger,
        UopConfig,
        UopDpConfig,
    )

    def chain(u, rep, nxt):
        u.repeat_count = rep
        u.trigger = (Trigger.SRC_TENSOR_DONE, Trigger.COUNT, Trigger.NONE)
        u.next_uop = (0, nxt, 0)
        u.require_inp0 = ENABLE
        u.require_inp1 = ENABLE
        return u

    # ---- 1x program: 1 elem/cycle, acc in block1, write once per group ----
    def u1x(kind, write):
        u = UopConfig()
        u.enable_input(InpSel.SRC_0, 0)  # x   -> block0 ALU A
        u.enable_input(InpSel.SRC_1, 1)  # w   -> block0 PREV_DELAY_0
        u.datapath_config[0] = UopDpConfig().enable_alu(
            AluOp.MULTIPLY, AluInp.PREV_ALU_OUT, AluInp.PREV_DELAY_0
        )
        op = AluOp.BYPASS if kind == "reset" else AluOp.ADD
        u.datapath_config[1] = UopDpConfig().enable_alu(
            op, AluInp.PREV_ALU_OUT, AluInp.CURR_ALU_OUT
        )
        for b in range(2, 8):
            u.datapath_config[b] = UopDpConfig().pass_through_alu()
        if write:
            u.enable_output(OutSel.ALU_OUT, OutPath.WR0_LO)
        return u

    # reset(1) -> acc(6) -> emit(1,write) -> reset(1) -> acc(6) -> emit -> ...
    ops_1x = [
        chain(u1x("reset", False), 1, 1),  # 0: entry reset
        chain(u1x("acc", False), 6, 2),    # 1
        chain(u1x("acc", True), 1, 3),     # 2: emit (8th element completes sum)
        chain(u1x("reset", False), 1, 4),  # 3: loop reset
        chain(u1x("acc", False), 6, 5),    # 4
        chain(u1x("acc", True), 1, 3),     # 5: emit -> loop reset
    ]

    # ---- 2x_1PORT program: 2 packed fp16/cycle, acc in block3 --------------
    def u2x(kind):
        u = UopConfig()
        u.enable_input(InpSel.SRC_0, 0)     # x_lo -> block0 ALU A
        u.enable_input(InpSel.SRC_1, 1)     # w_lo -> PREV_DELAY_0
        u.enable_input(InpSel.SRC_0_HI, 2)  # x_hi -> PREV_DELAY_1
        u.enable_input(InpSel.SRC_1_HI, 3)  # w_hi -> PREV_DELAY_2
        u.datapath_config[0] = (
            UopDpConfig()
            .enable_alu(AluOp.MULTIPLY, AluInp.PREV_ALU_OUT, AluInp.PREV_DELAY_0)
            .pass_through_delay(1, 2)
        )
        u.datapath_config[1] = (
            UopDpConfig()
            .enable_alu(AluOp.MULTIPLY, AluInp.PREV_DELAY_1, AluInp.PREV_DELAY_2)
            .enable_delay_from_src(DelayInp.PREV_ALU_OUT, 0)  # m_lo
        )
        u.datapath_config[2] = UopDpConfig().enable_alu(
            AluOp.ADD, AluInp.PREV_ALU_OUT, AluInp.PREV_DELAY_0  # t = m_hi+m_lo
        )
        b3 = UopDpConfig()
        if kind in ("reset", "reset_hold"):
            # drop the CURR feedback: acc <- t (new group's first pair)
            b3.enable_alu(AluOp.BYPASS, AluInp.PREV_ALU_OUT, AluInp.CURR_ALU_OUT)
            if kind == "reset_hold":
                # capture the completed even-group sum (block3's CURR flop)
                # into delay lane 4; later uops leave d4 disabled at block3,
                # so the flop HOLDS it until the emit element reads it.
                b3.enable_delay_from_src(DelayInp.CURR_ALU_OUT, 4)
        else:
            b3.enable_alu(AluOp.ADD, AluInp.PREV_ALU_OUT, AluInp.CURR_ALU_OUT)
        u.datapath_config[3] = b3
        for b in range(4, 8):
            cfg = UopDpConfig().pass_through_alu()
            if kind in ("emit", "acc", "reset_hold"):
                # keep the held even sum flowing toward the output flops
                cfg.pass_through_delay(4)
            u.datapath_config[b] = cfg
        if kind == "emit":
            # one packed 4B write per pair of groups: {lo=even sum, hi=odd sum}
            u.enable_output(OutSel.DELAY_4, OutPath.WR0_LO)
            u.enable_output(OutSel.ALU_OUT, OutPath.WR0_HI)
        return u

    # repeat_count counts ISSUE CYCLES (2 elements each in 2x mode):
    # resetE(1) -> accA(3) -> resetO(1, hold even sum) -> accB(2)
    #   -> emit(1, write pair) -> resetE(1) -> accA ...
    ops_2x = [
        chain(u2x("reset"), 1, 1),       # 0: entry (even group pair 0)
        chain(u2x("acc"), 3, 2),         # 1: even pairs 1-3
        chain(u2x("reset_hold"), 1, 3),  # 2: odd pair 0, park even sum in d4
        chain(u2x("acc"), 2, 4),         # 3: odd pairs 1-2
        chain(u2x("emit"), 1, 5),        # 4: odd pair 3, write {even,odd}
        chain(u2x("reset"), 1, 1),       # 5: loop reset (next even group)
    ]
    return ops_1x, ops_2x


def _register_pair_op():
    """Register SEGSUM8_PAIR2X_ANT (1x + 2x_1PORT programs, perf_max=1)."""
    import dataclasses

    from concourse import dve_ops
    from concourse.dve_spec import AluOp, Spec, Src0, Src1, scan
    from concourse.dve_uop import DveOpSpec

    for op in dve_ops.OPS:
        if op.name == _OP_NAME:
            return op

    def _ref(in0, in1, s0, s1, imm2):
        p = np.asarray(in0, np.float32) * np.asarray(in1, np.float32).reshape(
            np.asarray(in0).shape
        )
        return p.reshape(p.shape[0], -1, K).sum(axis=-1, dtype=np.float32)

    # body is structural only (Src1 presence -> rd1_en); semantics live in
    # the hand-built uop programs + `_ref` (used by the interpreter).
    spec = Spec(body=scan(AluOp.ADD, Src0 * Src1), reference=_ref)

    @dataclasses.dataclass(frozen=True)
    class _PairDveOp(dve_ops.DveOp):
        def compile(self, ver):
            key = (self.name, ver)
            cached = dve_ops._COMPILE_CACHE.get(key)
            if cached is not None:
                return cached
            u1x, u2x = _build_pair_uops()
            result = DveOpSpec(
                name=self.name,
                opcode=dve_ops.get_dve_sub_opcode(self.name),
                uops=u1x,
                uops_2x=u2x,
                perf_max=1,
                rd1_en=True,
            )
            result.validate(ver)
            dve_ops._COMPILE_CACHE[key] = result
            return result

    row = dve_ops._CUSTOM_DVE_ROW_BASE + len(dve_ops.OPS)
    op = _PairDveOp(_OP_NAME, spec, subdim=False, uops_sha={})
    dve_ops.OPS.append(op)
    dve_ops.CUSTOM_DVE_SPECS[_OP_NAME] = spec
    dve_ops._SUB_OPCODE_FOR_NAME[_OP_NAME] = row
    return op


def _build(b_shard, in_f, out_f, n_cores, x_bufs=4, quarters=2, tail_quarters=2,
           rpt=1):
    """Build + compile the per-core Bass module (SPMD across n_cores).

    rpt = x rows per partition in the steady-stream tiles.  rpt=2 makes
    each load a [128, 2, 8192] AP (4 MiB, 32 KiB contiguous per
    partition) - longer HBM bursts that hold rate better under
    paired-NC contention; w is then replicated rpt times along the free
    dim of wb so one scan instruction covers the whole tile stream.
    """
    from concourse import bacc, mybir, tile

    op = _register_pair_op()

    k = K
    n_blocks = b_shard // P
    qw = in_f // quarters  # quarter width (multiple of 16)
    f16 = mybir.dt.float16
    assert (n_blocks - 2) % rpt == 0, (n_blocks, rpt)

    nc = bacc.Bacc(
        "TRN2",
        target_bir_lowering=False,
        debug=False,
        enable_asserts=True,
        num_devices=n_cores,
    )
    x_d = nc.dram_tensor("x", [b_shard, in_f], f16, kind="ExternalInput")
    w_d = nc.dram_tensor("w", [1, in_f], f16, kind="ExternalInput")
    y_d = nc.dram_tensor("y", [b_shard, out_f], f16, kind="ExternalOutput")

    with tile.TileContext(nc) as tc:
        with (
            tc.tile_pool(name="consts", bufs=1) as cpool,
            tc.tile_pool(name="first", bufs=1) as fpool,
            tc.tile_pool(name="work", bufs=x_bufs) as pool,
            tc.tile_pool(name="outs", bufs=3) as ypool,
            tc.tile_pool(name="tailq", bufs=4) as qpool,
            tc.psum_pool(name="wpsum", bufs=2) as ppool,
        ):
            wb = cpool.tile([P, rpt * in_f], f16, tag="w")
            w_sb = cpool.tile([1, in_f], f16, tag="w_row")
            ones = cpool.tile([1, P], f16, tag="ones")

            def chunk(xap, w0, cw, y_ap, y_rearrange=None):
                """Grouped multiply-reduce of xap against wb[:, w0:w0+cw]."""
                cg = cw // k
                yt = ypool.tile([P, cg], f16, tag="s")
                ins = nc.vector._custom_dve(
                    op,
                    out=yt[:],
                    in0=xap,
                    in1=wb[:, w0 : w0 + cw],
                )
                ins.ins.perf_max = 1  # byte-36[7:6]: allow the 2x_1PORT slot
                src = yt[:]
                if y_rearrange is not None:
                    src = src.rearrange(y_rearrange, g=out_f)
                # y stores ride the ScalarE HWDGE queue so their semaphore
                # waits never block the x-load issue stream.
                nc.scalar.dma_start(out=y_ap, in_=src)

            # w broadcast [1, in_f] -> [128, rpt*in_f] OFF the HBM stream:
            # a 16 KiB w load rides first on the Sync FIFO, then the (idle)
            # PE replicates it into PSUM chunks via a ones[1,128] matmul
            # (K=1 contraction) and the (idle) ACT engine copies each PSUM
            # chunk to wb as fp16 (w repeated rpt times along the free
            # dim).  Saves the 2 MiB+ a host-replicated w load would cost.
            nc.sync.dma_start(out=w_sb[:], in_=w_d[:])
            nc.vector.memset(ones[:], 1.0)
            mm = 512   # PE moving-free-dim max; [P, 512] fp32 = 1 PSUM bank
            cw2 = 2048  # 4 banks per ACT copy: minimal op/sem traffic in the fill
            for c in range(rpt * in_f // cw2):
                pt = ppool.tile([P, cw2], mybir.dt.float32, tag="wp")
                for h in range(cw2 // mm):
                    s0 = (c * cw2 + h * mm) % in_f
                    nc.tensor.matmul(
                        pt[:, h * mm : (h + 1) * mm], ones[0:1, :],
                        w_sb[0:1, s0 : s0 + mm], start=True, stop=True,
                    )
                nc.scalar.copy(out=wb[:, c * cw2 : (c + 1) * cw2], in_=pt[:])

            # block 0 in quarters: quarter-scan q gates only on its own wb
            # range + x0 quarter, so scans start early.
            xt0 = fpool.tile([P, in_f], f16, tag="x0")
            for q in range(quarters):
                cs = slice(q * qw, (q + 1) * qw)
                nc.sync.dma_start(out=xt0[:, cs], in_=x_d[0:P, cs])
                chunk(xt0[:, cs], q * qw, qw, y_d[0:P, q * qw // k : (q + 1) * qw // k])

            # steady stream: blocks 1 .. n_blocks-2, rpt row-blocks per tile
            for r in range(P, (n_blocks - 1) * P, rpt * P):
                xt = pool.tile([P, rpt * in_f], f16, tag="x")
                nc.sync.dma_start(
                    out=xt[:].rearrange("p (t f) -> p t f", f=in_f),
                    in_=x_d[r : r + rpt * P, :].rearrange(
                        "(p t) f -> p t f", t=rpt
                    ),
                )
                chunk(
                    xt[:],
                    0,
                    rpt * in_f,
                    y_d[r : r + rpt * P, :].rearrange("(p t) g -> p t g", t=rpt),
                    y_rearrange="p (t g) -> p t g",
                )

            # split the final block so the post-stream tail is short
            rows = slice((n_blocks - 1) * P, n_blocks * P)
            tqw = in_f // tail_quarters
            for q in range(tail_quarters):
                xq = qpool.tile([P, tqw], f16, tag="xq")
                nc.sync.dma_start(
                    out=xq[:], in_=x_d[rows, q * tqw : (q + 1) * tqw]
                )
                chunk(xq[:], q * tqw, tqw,
                      y_d[rows, q * tqw // k : (q + 1) * tqw // k])
    nc.compile()
    return nc


def _prep_weights(log_weight):
    w = np.exp(np.asarray(log_weight, np.float64)).reshape(1, -1)  # [1, out_f*k]
    return np.ascontiguousarray(w.astype(np.float16))


def kernel(x, log_weight):
    from concourse import bass_utils

    x = np.ascontiguousarray(np.asarray(x).astype(np.float16))
    assert x.shape == (B, IN_F), x.shape
    b_shard = B // N_CORES

    if "nc" not in _CACHE:
        _CACHE["nc"] = _build(b_shard, IN_F, OUT_F, N_CORES)
    nc = _CACHE["nc"]

    wb = _prep_weights(log_weight)
    in_maps = [
        {"x": x[i * b_shard : (i + 1) * b_shard], "w": wb}
        for i in range(N_CORES)
    ]
    res = bass_utils.run_bass_kernel_spmd(nc, in_maps, core_ids=list(range(N_CORES)))
    y = np.concatenate([res.results[i]["y"] for i in range(N_CORES)], axis=0)
    return y.astype(np.float32)
